# revision 14
# baseline (speedup 1.0000x reference)
"""BiaffineSpanHead Trainium2 kernel.

Reference computation (B=4, S=1024, IN=1024, H=256, C=8):
    Hs = seq @ start_w.T + start_b            # [b, s, h]
    He = seq @ end_w.T + end_b                # [b, e, h]
    biaff[b,s,e,c] = sum_{h,g} Hs[b,s,h] U[h,c,g] He[b,e,g]
    out = biaff + ls[b,s,c] + le[b,e,c] + W_bias[c]
where ls = Hs @ Ws.T, le = He @ We.T  (Ws, We = W_weight split halves).

Sharding: 8 cores = (batch b, s-half). Each core computes out[b, s0:s0+512, :, :],
written c-major ([C, 512, 1024]) and transposed to [512, 1024, 8] on the host.

Per-core device algorithm (all matmul operands float32r):
    HsT[h, s]      = swT.T @ seqT_s   (+ start_b via eviction bias)
    HeT[h, e]      = ewT.T @ seqT_e   (+ end_b via eviction bias)
    TT[(c,g), s]   = U_flat.T @ HsT          (U_flat = U.reshape(H, C*H))
    R[:, c, e]     = broadcast of (le[e,c] + W_bias[c])   (K=1 ones matmuls)
    out[c, s, e]   = TT[c].T @ HeT  (+ ls[s,c] + R  fused into the PSUM eviction)
ls/le are computed on host via exact algebra: ls = seq @ (Ws@start_w).T + Ws@start_b.
"""

import numpy as np
import ml_dtypes

B, S, IN, H, C = 4, 1024, 1024, 256, 8
SL = S // 2          # s-slab per core
N_CORES = 8
P = 128              # partitions
NB = 512             # matmul free-dim block (one PSUM bank of fp32)
KT_IN = IN // P      # 8  k-tiles over IN
HC = H // P          # 2  chunks over H
NCH = C * H // P     # 16 chunks of TT
SC = SL // P         # 4  s-chunks per core
EB = S // NB         # 2  e-blocks

_cache = {}


def _build():
    import concourse.bacc as bacc
    import concourse.bass as bass
    import concourse.tile as tile
    import concourse.mybir as mybir

    f32 = mybir.dt.float32
    f32r = mybir.dt.float32r
    f16 = mybir.dt.float16
    bf16 = mybir.dt.bfloat16
    ADD = mybir.AluOpType.add

    nc = bacc.Bacc("TRN2", target_bir_lowering=False, debug=False, num_devices=N_CORES)

    seqT_e = nc.dram_tensor("seqT_e", [IN, S], bf16, kind="ExternalInput")
    seqT_s = nc.dram_tensor("seqT_s", [IN, SL], bf16, kind="ExternalInput")
    u = nc.dram_tensor("u", [H, C * H], bf16, kind="ExternalInput")
    swT = nc.dram_tensor("swT", [IN, H], bf16, kind="ExternalInput")
    ewT = nc.dram_tensor("ewT", [IN, H], bf16, kind="ExternalInput")
    sbb = nc.dram_tensor("sbb", [P, HC], f32, kind="ExternalInput")
    ebb = nc.dram_tensor("ebb", [P, HC], f32, kind="ExternalInput")
    lsb = nc.dram_tensor("lsb", [P, SC * C], f32, kind="ExternalInput")
    let4 = nc.dram_tensor("let4", [4, C * S // 4], bf16, kind="ExternalInput")
    ones1 = nc.dram_tensor("ones1", [P, P], bf16, kind="ExternalInput")
    out = nc.dram_tensor("out", [C, SL, S], f16, kind="ExternalOutput")

    LROW = C * S // 4  # 2048 values per let4 row

    with tile.TileContext(nc) as tc:
        with (
            tc.tile_pool(name="inp", bufs=1) as inp,
            tc.tile_pool(name="mid", bufs=1) as mid,
            tc.tile_pool(name="outp", bufs=4) as outp,
            tc.tile_pool(name="pp", bufs=2, space="PSUM") as pp,
            tc.tile_pool(name="rp", bufs=1, space="PSUM") as rp,
            tc.tile_pool(name="pb", bufs=5, space="PSUM") as pb,
        ):
            # ---- input tiles ----
            swT_t = inp.tile([P, KT_IN, H], bf16, tag="swT")
            seqs_t = inp.tile([P, KT_IN, SL], bf16, tag="seqs")
            u_t = inp.tile([P, HC, C * H], bf16, tag="u")
            ewT_t = inp.tile([P, KT_IN, H], bf16, tag="ewT")
            seqe_t = inp.tile([P, KT_IN, S], bf16, tag="seqe")
            sbb_t = inp.tile([P, HC], f32, tag="sbb")
            ebb_t = inp.tile([P, HC], f32, tag="ebb")
            lsb_t = inp.tile([P, SC, C], f32, tag="lsb")
            let_t = inp.tile([P, LROW], bf16, tag="let")
            ones_t = inp.tile([P, P], bf16, tag="ones1")

            dma = nc.sync.dma_start  # input loads on the SP HWDGE ring (SP is otherwise idle)
            dma(ones_t[:], ones1.ap())
            dma(
                let_t[:].rearrange("(a b) x -> a b x", b=32)[:, 0, :],
                let4.ap(),
            )
            dma(sbb_t[:], sbb.ap())
            dma(ebb_t[:], ebb.ap())
            dma(lsb_t[:], lsb.ap().rearrange("p (a c) -> p a c", c=C))
            dma(swT_t[:], swT.ap().rearrange("(k p) h -> p k h", p=P))
            seqs_r = seqT_s.ap().rearrange("(k p) s -> p k s", p=P)
            for half in range(2):
                dma(
                    seqs_t[:, half * (KT_IN // 2):(half + 1) * (KT_IN // 2), :],
                    seqs_r[:, half * (KT_IN // 2):(half + 1) * (KT_IN // 2), :],
                )
            dma(u_t[:], u.ap().rearrange("(k p) m -> p k m", p=P))
            dma(ewT_t[:], ewT.ap().rearrange("(k p) h -> p k h", p=P))
            seqe_r = seqT_e.ap().rearrange("(k p) s -> p k s", p=P)
            for eb in range(EB):
                dma(seqe_t[:, :, eb * NB:(eb + 1) * NB], seqe_r[:, :, eb * NB:(eb + 1) * NB])

            # ---- intermediate tiles ----
            hsT_t = mid.tile([P, HC, SL], bf16, tag="hsT")
            heT_t = mid.tile([P, HC, S], bf16, tag="heT")
            tt_t = mid.tile([P, NCH, SL], bf16, tag="tt")
            r_t = mid.tile([P, C, S], f32, tag="r")

            # R chunks (le broadcast) are interleaved into the pre-stage below;
            # each is one K=1 ones-matmul + ACT eviction on its own PSUM tag.
            r_chunks = [divmod(i, EB) for i in range(C * EB)]

            def emit_r_chunk():
                if not r_chunks:
                    return
                c, eb = r_chunks.pop(0)
                f = c * S + eb * NB
                row, off = divmod(f, LROW)
                ps = rp.tile([P, NB], f32, tag="rps")
                nc.tensor.matmul(
                    ps[:],
                    ones_t[32 * row:32 * row + 1, :],
                    let_t[32 * row:32 * row + 1, off:off + NB],
                    start=True,
                    stop=True,
                    tile_position=(32 * row, 0),
                )
                nc.scalar.copy(r_t[:, c, eb * NB:(eb + 1) * NB], ps[:])

            # ---- stage 1: HsT[h, s] = swT.T @ seqT_s  (+ start_b) ----
            for hc in range(HC):
                ps = pp.tile([P, SL], f32, tag="pre")
                for kt in range(KT_IN):
                    nc.tensor.matmul(
                        ps[:],
                        swT_t[:, kt, hc * P:(hc + 1) * P],
                        seqs_t[:, kt, :],
                        start=(kt == 0),
                        stop=(kt == KT_IN - 1),
                    )
                nc.scalar.add(hsT_t[:, hc, :], ps[:], sbb_t[:, hc:hc + 1])
                emit_r_chunk()

            # ---- stage 2: TT[(c,g), s] = U_flat.T @ HsT ----
            for ch in range(NCH):
                ps = pp.tile([P, SL], f32, tag="pre")
                for hc in range(HC):
                    nc.tensor.matmul(
                        ps[:],
                        u_t[:, hc, ch * P:(ch + 1) * P],
                        hsT_t[:, hc, :],
                        start=(hc == 0),
                        stop=(hc == HC - 1),
                    )
                if ch % 2 == 0:
                    nc.vector.tensor_copy(tt_t[:, ch, :], ps[:])
                else:
                    nc.scalar.copy(tt_t[:, ch, :], ps[:])
                emit_r_chunk()

            # ---- stage 3: HeT[h, e] = ewT.T @ seqT_e  (+ end_b) ----
            for eb in range(EB):
                for hc in range(HC):
                    ps = pp.tile([P, NB], f32, tag="pre")
                    for kt in range(KT_IN):
                        nc.tensor.matmul(
                            ps[:],
                            ewT_t[:, kt, hc * P:(hc + 1) * P],
                            seqe_t[:, kt, eb * NB:(eb + 1) * NB],
                            start=(kt == 0),
                            stop=(kt == KT_IN - 1),
                        )
                    nc.scalar.add(heT_t[:, hc, eb * NB:(eb + 1) * NB], ps[:], ebb_t[:, hc:hc + 1])

            while r_chunks:
                emit_r_chunk()

            # ---- stage 4: biaffine, fused linear term in eviction ----
            out_r = out.ap().rearrange("c (a p) (b e) -> a b p c e", p=P, e=NB)
            for sc in range(SC):
                for eb in range(EB):
                    ot = outp.tile([P, C, NB], f16, tag="ot")
                    for c in range(C):
                        on_act = c in (3, 7)
                        ps = pb.tile([P, NB], f32, tag="bia")
                        for gt in range(HC):
                            nc.tensor.matmul(
                                ps[:],
                                tt_t[:, c * HC + gt, sc * P:(sc + 1) * P],
                                heT_t[:, gt, eb * NB:(eb + 1) * NB],
                                start=(gt == 0),
                                stop=(gt == HC - 1) and not on_act,
                            )
                        if on_act:
                            # fold the le-term via a K=1 ones matmul, ls via eviction bias
                            f = c * S + eb * NB
                            row, off = divmod(f, LROW)
                            nc.tensor.matmul(
                                ps[:],
                                ones_t[32 * row:32 * row + 1, :],
                                let_t[32 * row:32 * row + 1, off:off + NB],
                                start=False,
                                stop=True,
                                tile_position=(32 * row, 0),
                            )
                            nc.scalar.add(ot[:, c, :], ps[:], lsb_t[:, sc, c:c + 1])
                        else:
                            nc.vector.scalar_tensor_tensor(
                                out=ot[:, c, :],
                                in0=ps[:],
                                scalar=lsb_t[:, sc, c:c + 1],
                                in1=r_t[:, c, eb * NB:(eb + 1) * NB],
                                op0=ADD,
                                op1=ADD,
                            )
                    nc.sync.dma_start(out_r[sc, eb], ot[:])

    nc.compile()
    return nc


def _prep_inputs(seq_feats, U, W_weight, W_bias, start_w, start_b, end_w, end_b):
    f = np.float32
    seq = np.asarray(seq_feats, f)
    U = np.asarray(U, f)
    W_weight = np.asarray(W_weight, f)
    W_bias = np.asarray(W_bias, f)
    start_w = np.asarray(start_w, f)
    start_b = np.asarray(start_b, f)
    end_w = np.asarray(end_w, f)
    end_b = np.asarray(end_b, f)

    Ws, We = W_weight[:, :H], W_weight[:, H:]
    # exact algebra: ls = Hs @ Ws.T = seq @ (Ws@start_w).T + Ws@start_b
    ls = seq @ (Ws @ start_w).T + Ws @ start_b           # [B, S, C]
    le = seq @ (We @ end_w).T + (We @ end_b + W_bias)    # [B, S, C]

    bf = ml_dtypes.bfloat16
    u_flat = np.ascontiguousarray(U.reshape(H, C * H)).astype(bf)
    swT = np.ascontiguousarray(start_w.T).astype(bf)
    ewT = np.ascontiguousarray(end_w.T).astype(bf)
    sbb = np.ascontiguousarray(start_b.reshape(HC, P).T)
    ebb = np.ascontiguousarray(end_b.reshape(HC, P).T)
    seqT = np.ascontiguousarray(seq.transpose(0, 2, 1)).astype(bf)  # [B, IN, S]
    ones1 = np.ones((P, P), ml_dtypes.bfloat16)

    in_maps = []
    for core in range(N_CORES):
        b, sh = divmod(core, 2)
        s0 = sh * SL
        lsb = np.ascontiguousarray(
            ls[b, s0:s0 + SL, :].reshape(SC, P, C).transpose(1, 0, 2).reshape(P, SC * C)
        )
        let4 = np.ascontiguousarray(le[b].T).reshape(4, C * S // 4).astype(ml_dtypes.bfloat16)
        in_maps.append(
            {
                "seqT_e": seqT[b],
                "seqT_s": np.ascontiguousarray(seqT[b, :, s0:s0 + SL]),
                "u": u_flat,
                "swT": swT,
                "ewT": ewT,
                "sbb": sbb,
                "ebb": ebb,
                "lsb": lsb,
                "let4": let4,
                "ones1": ones1,
            }
        )
    return in_maps


def _run(in_maps, trace=False):
    from concourse.bass_utils import run_bass_kernel_spmd

    if "nc" not in _cache:
        _cache["nc"] = _build()
    kwargs = {}
    if trace:
        kwargs = dict(trace=True, trace_cores=list(range(N_CORES)))
    return run_bass_kernel_spmd(
        _cache["nc"], in_maps, core_ids=list(range(N_CORES)), **kwargs
    )


def kernel(seq_feats, U, W_weight, W_bias, start_w, start_b, end_w, end_b, _trace=False):
    in_maps = _prep_inputs(
        seq_feats, U, W_weight, W_bias, start_w, start_b, end_w, end_b
    )
    res = _run(in_maps, trace=_trace)
    full = np.empty((B, S, S, C), np.float32)
    for core in range(N_CORES):
        b, sh = divmod(core, 2)
        s0 = sh * SL
        full[b, s0:s0 + SL] = res.results[core]["out"].transpose(1, 2, 0).astype(np.float32)
    if _trace:
        kernel.last_result = res
    return full


# revision 15
# speedup vs baseline: 1.0488x; 1.0488x over previous
"""BiaffineSpanHead Trainium2 kernel.

Reference computation (B=4, S=1024, IN=1024, H=256, C=8):
    Hs = seq @ start_w.T + start_b            # [b, s, h]
    He = seq @ end_w.T + end_b                # [b, e, h]
    biaff[b,s,e,c] = sum_{h,g} Hs[b,s,h] U[h,c,g] He[b,e,g]
    out = biaff + ls[b,s,c] + le[b,e,c] + W_bias[c]
where ls = Hs @ Ws.T, le = He @ We.T  (Ws, We = W_weight split halves).

Sharding: 8 cores = (batch b, s-half). Each core computes out[b, s0:s0+512, :, :],
written c-major ([C, 512, 1024]) and transposed to [512, 1024, 8] on the host.

Per-core device algorithm (all matmul operands float32r):
    HsT[h, s]      = swT.T @ seqT_s   (+ start_b via eviction bias)
    HeT[h, e]      = ewT.T @ seqT_e   (+ end_b via eviction bias)
    TT[(c,g), s]   = U_flat.T @ HsT          (U_flat = U.reshape(H, C*H))
    R[:, c, e]     = broadcast of (le[e,c] + W_bias[c])   (K=1 ones matmuls)
    out[c, s, e]   = TT[c].T @ HeT  (+ ls[s,c] + R  fused into the PSUM eviction)
ls/le are computed on host via exact algebra: ls = seq @ (Ws@start_w).T + Ws@start_b.
"""

import numpy as np
import ml_dtypes

B, S, IN, H, C = 4, 1024, 1024, 256, 8
SL = S // 2          # s-slab per core
N_CORES = 8
P = 128              # partitions
NB = 512             # matmul free-dim block (one PSUM bank of fp32)
KT_IN = IN // P      # 8  k-tiles over IN
HC = H // P          # 2  chunks over H
NCH = C * H // P     # 16 chunks of TT
SC = SL // P         # 4  s-chunks per core
EB = S // NB         # 2  e-blocks

_cache = {}


def _build():
    import concourse.bacc as bacc
    import concourse.bass as bass
    import concourse.tile as tile
    import concourse.mybir as mybir

    f32 = mybir.dt.float32
    f32r = mybir.dt.float32r
    f16 = mybir.dt.float16
    bf16 = mybir.dt.bfloat16
    ADD = mybir.AluOpType.add

    nc = bacc.Bacc("TRN2", target_bir_lowering=False, debug=False, num_devices=N_CORES)

    seqT_e = nc.dram_tensor("seqT_e", [IN, S], bf16, kind="ExternalInput")
    seqT_s = nc.dram_tensor("seqT_s", [IN, SL], bf16, kind="ExternalInput")
    u = nc.dram_tensor("u", [H, C * H], bf16, kind="ExternalInput")
    swT = nc.dram_tensor("swT", [IN, H], bf16, kind="ExternalInput")
    ewT = nc.dram_tensor("ewT", [IN, H], bf16, kind="ExternalInput")
    sbb = nc.dram_tensor("sbb", [P, HC], f32, kind="ExternalInput")
    ebb = nc.dram_tensor("ebb", [P, HC], f32, kind="ExternalInput")
    lsb = nc.dram_tensor("lsb", [P, SC * C], f32, kind="ExternalInput")
    let4 = nc.dram_tensor("let4", [4, C * S // 4], bf16, kind="ExternalInput")
    ones1 = nc.dram_tensor("ones1", [P, P], bf16, kind="ExternalInput")
    out = nc.dram_tensor("out", [C, SL, S], f16, kind="ExternalOutput")

    LROW = C * S // 4  # 2048 values per let4 row

    with tile.TileContext(nc) as tc:
        with (
            tc.tile_pool(name="inp", bufs=1) as inp,
            tc.tile_pool(name="mid", bufs=1) as mid,
            tc.tile_pool(name="outp", bufs=4) as outp,
            tc.tile_pool(name="pp", bufs=3, space="PSUM") as pp,
            tc.tile_pool(name="pb", bufs=5, space="PSUM") as pb,
        ):
            # ---- input tiles ----
            swT_t = inp.tile([P, KT_IN, H], bf16, tag="swT")
            seqs_t = inp.tile([P, KT_IN, SL], bf16, tag="seqs")
            u_t = inp.tile([P, HC, C * H], bf16, tag="u")
            ewT_t = inp.tile([P, KT_IN, H], bf16, tag="ewT")
            seqe_t = inp.tile([P, KT_IN, S], bf16, tag="seqe")
            sbb_t = inp.tile([P, HC], f32, tag="sbb")
            ebb_t = inp.tile([P, HC], f32, tag="ebb")
            lsb_t = inp.tile([P, SC, C], f32, tag="lsb")
            let_t = inp.tile([P, LROW], bf16, tag="let")
            ones_t = inp.tile([P, P], bf16, tag="ones1")

            dma = nc.sync.dma_start  # input loads on the SP HWDGE ring (SP is otherwise idle)
            dma(ones_t[:], ones1.ap())
            dma(
                let_t[:].rearrange("(a b) x -> a b x", b=32)[:, 0, :],
                let4.ap(),
            )
            dma(sbb_t[:], sbb.ap())
            dma(ebb_t[:], ebb.ap())
            dma(lsb_t[:], lsb.ap().rearrange("p (a c) -> p a c", c=C))
            dma(swT_t[:], swT.ap().rearrange("(k p) h -> p k h", p=P))
            seqs_r = seqT_s.ap().rearrange("(k p) s -> p k s", p=P)
            for half in range(2):
                dma(
                    seqs_t[:, half * (KT_IN // 2):(half + 1) * (KT_IN // 2), :],
                    seqs_r[:, half * (KT_IN // 2):(half + 1) * (KT_IN // 2), :],
                )
            dma(u_t[:], u.ap().rearrange("(k p) m -> p k m", p=P))
            dma(ewT_t[:], ewT.ap().rearrange("(k p) h -> p k h", p=P))
            seqe_r = seqT_e.ap().rearrange("(k p) s -> p k s", p=P)
            for eb in range(EB):
                dma(seqe_t[:, :, eb * NB:(eb + 1) * NB], seqe_r[:, :, eb * NB:(eb + 1) * NB])

            # ---- intermediate tiles ----
            hsT_t = mid.tile([P, HC, SL], bf16, tag="hsT")
            heT_t = mid.tile([P, HC, S], bf16, tag="heT")
            tt_t = mid.tile([P, NCH, SL], bf16, tag="tt")
            r_t = mid.tile([P, C, S], f32, tag="r")

            # ---- stage 0: R[:, c, e] = broadcast(le[e, c] + W_bias[c]) ----
            for c in range(C):
                for eb in range(EB):
                    f = c * S + eb * NB
                    row, off = divmod(f, LROW)
                    ps = pp.tile([P, NB], f32, tag="pre")
                    nc.tensor.matmul(
                        ps[:],
                        ones_t[32 * row:32 * row + 1, :],
                        let_t[32 * row:32 * row + 1, off:off + NB],
                        start=True,
                        stop=True,
                        tile_position=(32 * row, 0),
                    )
                    nc.scalar.copy(r_t[:, c, eb * NB:(eb + 1) * NB], ps[:])

            # ---- stage 1: HsT[h, s] = swT.T @ seqT_s  (+ start_b) ----
            for hc in range(HC):
                ps = pp.tile([P, SL], f32, tag="pre")
                for kt in range(KT_IN):
                    nc.tensor.matmul(
                        ps[:],
                        swT_t[:, kt, hc * P:(hc + 1) * P],
                        seqs_t[:, kt, :],
                        start=(kt == 0),
                        stop=(kt == KT_IN - 1),
                    )
                nc.scalar.add(hsT_t[:, hc, :], ps[:], sbb_t[:, hc:hc + 1])

            # ---- stage 2: TT[(c,g), s] = U_flat.T @ HsT ----
            for ch in range(NCH):
                ps = pp.tile([P, SL], f32, tag="pre")
                for hc in range(HC):
                    nc.tensor.matmul(
                        ps[:],
                        u_t[:, hc, ch * P:(ch + 1) * P],
                        hsT_t[:, hc, :],
                        start=(hc == 0),
                        stop=(hc == HC - 1),
                    )
                nc.scalar.copy(tt_t[:, ch, :], ps[:])

            # ---- stage 3: HeT[h, e] = ewT.T @ seqT_e  (+ end_b) ----
            for eb in range(EB):
                for hc in range(HC):
                    ps = pp.tile([P, NB], f32, tag="pre")
                    for kt in range(KT_IN):
                        nc.tensor.matmul(
                            ps[:],
                            ewT_t[:, kt, hc * P:(hc + 1) * P],
                            seqe_t[:, kt, eb * NB:(eb + 1) * NB],
                            start=(kt == 0),
                            stop=(kt == KT_IN - 1),
                        )
                    nc.scalar.add(heT_t[:, hc, eb * NB:(eb + 1) * NB], ps[:], ebb_t[:, hc:hc + 1])

            # ---- stage 4: biaffine, fused linear term in eviction ----
            out_r = out.ap().rearrange("c (a p) (b e) -> a b p c e", p=P, e=NB)
            for sc in range(SC):
                for eb in range(EB):
                    ot = outp.tile([P, C, NB], f16, tag="ot")
                    for c in range(C):
                        on_act = c in (3, 7)
                        ps = pb.tile([P, NB], f32, tag="bia")
                        for gt in range(HC):
                            nc.tensor.matmul(
                                ps[:],
                                tt_t[:, c * HC + gt, sc * P:(sc + 1) * P],
                                heT_t[:, gt, eb * NB:(eb + 1) * NB],
                                start=(gt == 0),
                                stop=(gt == HC - 1) and not on_act,
                            )
                        if on_act:
                            # fold the le-term via a K=1 ones matmul, ls via eviction bias
                            f = c * S + eb * NB
                            row, off = divmod(f, LROW)
                            nc.tensor.matmul(
                                ps[:],
                                ones_t[32 * row:32 * row + 1, :],
                                let_t[32 * row:32 * row + 1, off:off + NB],
                                start=False,
                                stop=True,
                                tile_position=(32 * row, 0),
                            )
                            nc.scalar.add(ot[:, c, :], ps[:], lsb_t[:, sc, c:c + 1])
                        else:
                            nc.vector.scalar_tensor_tensor(
                                out=ot[:, c, :],
                                in0=ps[:],
                                scalar=lsb_t[:, sc, c:c + 1],
                                in1=r_t[:, c, eb * NB:(eb + 1) * NB],
                                op0=ADD,
                                op1=ADD,
                            )
                    nc.sync.dma_start(out_r[sc, eb], ot[:])

    nc.compile()
    return nc


def _prep_inputs(seq_feats, U, W_weight, W_bias, start_w, start_b, end_w, end_b):
    f = np.float32
    seq = np.asarray(seq_feats, f)
    U = np.asarray(U, f)
    W_weight = np.asarray(W_weight, f)
    W_bias = np.asarray(W_bias, f)
    start_w = np.asarray(start_w, f)
    start_b = np.asarray(start_b, f)
    end_w = np.asarray(end_w, f)
    end_b = np.asarray(end_b, f)

    Ws, We = W_weight[:, :H], W_weight[:, H:]
    # exact algebra: ls = Hs @ Ws.T = seq @ (Ws@start_w).T + Ws@start_b
    ls = seq @ (Ws @ start_w).T + Ws @ start_b           # [B, S, C]
    le = seq @ (We @ end_w).T + (We @ end_b + W_bias)    # [B, S, C]

    bf = ml_dtypes.bfloat16
    u_flat = np.ascontiguousarray(U.reshape(H, C * H)).astype(bf)
    swT = np.ascontiguousarray(start_w.T).astype(bf)
    ewT = np.ascontiguousarray(end_w.T).astype(bf)
    sbb = np.ascontiguousarray(start_b.reshape(HC, P).T)
    ebb = np.ascontiguousarray(end_b.reshape(HC, P).T)
    seqT = np.ascontiguousarray(seq.transpose(0, 2, 1)).astype(bf)  # [B, IN, S]
    ones1 = np.ones((P, P), ml_dtypes.bfloat16)

    in_maps = []
    for core in range(N_CORES):
        b, sh = divmod(core, 2)
        s0 = sh * SL
        lsb = np.ascontiguousarray(
            ls[b, s0:s0 + SL, :].reshape(SC, P, C).transpose(1, 0, 2).reshape(P, SC * C)
        )
        let4 = np.ascontiguousarray(le[b].T).reshape(4, C * S // 4).astype(ml_dtypes.bfloat16)
        in_maps.append(
            {
                "seqT_e": seqT[b],
                "seqT_s": np.ascontiguousarray(seqT[b, :, s0:s0 + SL]),
                "u": u_flat,
                "swT": swT,
                "ewT": ewT,
                "sbb": sbb,
                "ebb": ebb,
                "lsb": lsb,
                "let4": let4,
                "ones1": ones1,
            }
        )
    return in_maps


def _run(in_maps, trace=False):
    from concourse.bass_utils import run_bass_kernel_spmd

    if "nc" not in _cache:
        _cache["nc"] = _build()
    kwargs = {}
    if trace:
        kwargs = dict(trace=True, trace_cores=list(range(N_CORES)))
    return run_bass_kernel_spmd(
        _cache["nc"], in_maps, core_ids=list(range(N_CORES)), **kwargs
    )


def kernel(seq_feats, U, W_weight, W_bias, start_w, start_b, end_w, end_b, _trace=False):
    in_maps = _prep_inputs(
        seq_feats, U, W_weight, W_bias, start_w, start_b, end_w, end_b
    )
    res = _run(in_maps, trace=_trace)
    full = np.empty((B, S, S, C), np.float32)
    for core in range(N_CORES):
        b, sh = divmod(core, 2)
        s0 = sh * SL
        full[b, s0:s0 + SL] = res.results[core]["out"].transpose(1, 2, 0).astype(np.float32)
    if _trace:
        kernel.last_result = res
    return full
